# revision 16
# baseline (speedup 1.0000x reference)
"""Masked cross-entropy loss (ragged sequences) on 8 Trainium2 NeuronCores.

loss = sum_{valid} (logsumexp_v(logits[b,s,:]) - logits[b,s,tgt]) / n_valid
where valid = (position k < lengths[b]) & (tgt != 0), logits = output[:, 1:].

The heavy work is the per-token sum_v exp(x_v) over the 32000-wide vocab.
Valid tokens are packed host-side and shipped as fp8-e4m3 (halves HBM
traffic); the vocab is column-split across two concurrent engine paths per
core so no single engine is the bottleneck:

  * ACT path (V_A cols, full 128-token tiles): ScalarE activation Exp with
    per-partition accumulate, layout [tokens(128p), vocab(free)].
  * DVE+PE path (V_B cols, all tokens; plus the partial-tile tail tokens'
    V_A cols): layout [vocab(128p), tokens(free)]. VectorE computes a
    Schraudolph bit-trick exp in ONE tensor_scalar pass:
    i16 = round(x*1024/ln2 + (15*1024 + C)); those bits reinterpreted as
    fp16 ARE ~exp(x) (C calibrated to make the mean ratio 1). TensorE sums
    over the 128 vocab partitions with a ones[128,1] stationary matmul
    accumulating into PSUM[1, tokens] across all vocab chunks.

Host adds the two partial sums per token, takes log, gathers target logits
from the f32 data, masks (ignore_index=0) and reduces. Device traffic is
n_tokens*32000 bytes/core, streamed via a few dozen fat contiguous DMAs.
"""

import numpy as np

B, SP1, V = 16, 513, 32000
S = SP1 - 1
NCORES = 8
P = 128

V_A = 16512                # ACT-path vocab columns (when full tiles exist)
ACT_RAMP = [1536, 2048, 2560, 3072, 3648, 3648]   # tile-0 chunks, sum V_A
ACT_CW = 8256              # steady-tile chunk width (2 per tile)
CPB = 7                    # vocab chunks per DVE/DMA block (xb path)
CPB_C = 32                 # chunks per block for the tail (xc) path
PSUM_FD = 512

TRICK_A = 1024.0 / float(np.log(2.0))
TRICK_B = 1024.0 * 15 - 59.143        # exponent bias + calibration

_programs = {}             # (T_f, n_real) -> compiled Bacc program


def _geom(T_f, n_real):
    va = V_A if T_f > 0 else 0
    nch_b = (V - va) // P                  # V - va is a multiple of 128
    tail = n_real - T_f * P
    nch_c = va // P if tail else 0
    return va, nch_b, tail, nch_c


def _build_program(T_f, n_real):
    import concourse.bacc as bacc
    import concourse.tile as tile
    from concourse import mybir

    va, nch_b, tail, nch_c = _geom(T_f, n_real)
    assert T_f * P + tail == n_real and 0 <= tail < P

    nc = bacc.Bacc("TRN2", target_bir_lowering=False, debug=False,
                   num_devices=NCORES)
    xb = nc.dram_tensor("xb", [P, nch_b * n_real], mybir.dt.float8e4,
                        kind="ExternalInput").ap()
    xa = xc = sa = None
    if T_f:
        xa = nc.dram_tensor("xa", [T_f * P, va], mybir.dt.float8e4,
                            kind="ExternalInput").ap()
        sa = nc.dram_tensor("sa", [P, T_f], mybir.dt.float32,
                            kind="ExternalOutput").ap()
    if nch_c:
        xc = nc.dram_tensor("xc", [P, nch_c * tail], mybir.dt.float8e4,
                            kind="ExternalInput").ap()
    sb = nc.dram_tensor("sb", [1, n_real], mybir.dt.float32,
                        kind="ExternalOutput").ap()

    g1 = min(n_real, PSUM_FD)
    groups = [(0, g1)] + ([(g1, n_real - g1)] if g1 < n_real else [])
    assert sum(ACT_RAMP) == V_A and 2 * ACT_CW == V_A
    rampdn = [8256, 4128, 2560, 1568]          # small tail -> short pipe drain
    assert sum(rampdn) == V_A
    sched = [ACT_RAMP if j == 0 else
             (rampdn if j == T_f - 1 and T_f > 1 else [ACT_CW, ACT_CW])
             for j in range(T_f)]
    # matmul counts per PSUM group, for start/stop flags
    writes = [nch_b + (nch_c if g0 + gw > T_f * P else 0)
              for (g0, gw) in groups]
    seen = [0] * len(groups)

    with tile.TileContext(nc) as tc:
        with (
            tc.tile_pool(name="xpa", bufs=4) as xpa,
            tc.tile_pool(name="scr", bufs=2) as scr,
            tc.tile_pool(name="sm", bufs=2) as sm,
            tc.tile_pool(name="xpb", bufs=6) as xpb,
            tc.tile_pool(name="itp", bufs=4) as itp,
            tc.tile_pool(name="one", bufs=1) as one,
            tc.tile_pool(name="ps", bufs=1, space="PSUM") as ps,
        ):
            ones = one.tile([P, 1], mybir.dt.float16)
            nc.vector.memset(ones, 1.0)
            totalA = one.tile([P, max(T_f, 1)], mybir.dt.float32)
            seB_sb = one.tile([1, n_real], mybir.dt.float32)
            psums = [ps.tile([1, gw], mybir.dt.float32, tag=f"ps{g}",
                             name=f"psum{g}")
                     for g, (_, gw) in enumerate(groups)]

            def mm(g, rhs, goff, gwid):
                """Accumulating matmul into group g's psum tile."""
                seen[g] += 1
                nc.tensor.matmul(psums[g][:, goff:goff + gwid], ones, rhs,
                                 start=(seen[g] == 1),
                                 stop=(seen[g] == writes[g]))

            def act_steps():
                for j in range(T_f):
                    chunks = sched[j]
                    sums = sm.tile([P, len(chunks)], mybir.dt.float32,
                                   tag="sums")
                    off = 0
                    for c, cw in enumerate(chunks):
                        xt = xpa.tile([P, cw], mybir.dt.float8e4, tag="xt")
                        nc.sync.dma_start(
                            out=xt, in_=xa[j * P:(j + 1) * P,
                                           off:off + cw])
                        et = scr.tile([P, cw], mybir.dt.float8e4, tag="scr")
                        nc.scalar.activation(
                            et, xt, mybir.ActivationFunctionType.Exp,
                            accum_out=sums[:, c:c + 1])
                        off += cw
                        yield cw * P
                    # ACT reduces its own per-chunk sums (Relu == identity on
                    # positive values, same table set as Exp) so the reduce
                    # never crosses engines and cannot stall the DVE queue.
                    rscr = sm.tile([P, len(chunks)], mybir.dt.float32,
                                   tag="rscr")
                    nc.scalar.activation(
                        rscr, sums, mybir.ActivationFunctionType.Relu,
                        accum_out=totalA[:, j:j + 1])
                    if j == T_f - 1:
                        nc.sync.dma_start(out=sa, in_=totalA)

            def xb_block(b0):
                k = min(CPB, nch_b - b0)
                xt = xpb.tile([P, k * n_real], mybir.dt.float8e4, tag="xb")
                nc.sync.dma_start(
                    out=xt, in_=xb[:, b0 * n_real:(b0 + k) * n_real])
                it = itp.tile([P, k * n_real], mybir.dt.int16, tag="it")
                nc.vector.tensor_scalar(
                    it, xt, TRICK_A, TRICK_B,
                    mybir.AluOpType.mult, mybir.AluOpType.add)
                ft = it.bitcast(mybir.dt.float16)
                for i in range(k):
                    for g, (g0, gw) in enumerate(groups):
                        mm(g, ft[:, i * n_real + g0:i * n_real + g0 + gw],
                           0, gw)
                return k * n_real * P

            def xc_block(b0):
                # tail tokens' ACT-share columns, [vocab(128p), tail] chunks
                k = min(CPB_C, nch_c - b0)
                xt = xpb.tile([P, k * tail], mybir.dt.float8e4, tag="xc")
                nc.sync.dma_start(
                    out=xt, in_=xc[:, b0 * tail:(b0 + k) * tail])
                it = itp.tile([P, k * tail], mybir.dt.int16, tag="itc")
                nc.vector.tensor_scalar(
                    it, xt, TRICK_A, TRICK_B,
                    mybir.AluOpType.mult, mybir.AluOpType.add)
                ft = it.bitcast(mybir.dt.float16)
                goff = T_f * P - groups[-1][0]
                for i in range(k):
                    mm(len(groups) - 1, ft[:, i * tail:(i + 1) * tail],
                       goff, tail)
                return k * tail * P

            def dve_steps():
                # interleave the small xc blocks evenly among the xb blocks
                items = [("b", b0) for b0 in range(0, nch_b, CPB)]
                nb = len(items)
                ncb = -(-nch_c // CPB_C) if nch_c else 0
                for i in range(ncb, 0, -1):
                    items.insert(i * nb // (ncb + 1) + 1,
                                 ("c", (i - 1) * CPB_C))
                for kind, b0 in items:
                    yield xb_block(b0) if kind == "b" else xc_block(b0)

            ga, gb = act_steps(), dve_steps()
            ba = bb = 0.0
            a_done = b_done = False
            while not (a_done and b_done):
                if b_done or (not a_done and ba <= bb):
                    try:
                        ba += next(ga)
                    except StopIteration:
                        a_done = True
                else:
                    try:
                        bb += next(gb)
                    except StopIteration:
                        b_done = True

            for g, (g0, gw) in enumerate(groups):
                nc.vector.tensor_copy(seB_sb[:, g0:g0 + gw], psums[g])
            nc.sync.dma_start(out=sb, in_=seB_sb)

    nc.compile()
    return nc


def _get_program(T_f, n_real):
    key = (T_f, n_real)
    if key not in _programs:
        _programs[key] = _build_program(T_f, n_real)
    return _programs[key]


def _blockify(arr2d, nch, cpb, fd):
    """[nch*P, fd] -> [P, nch*fd] with cpb-chunk blocks laid contiguously."""
    a = np.ascontiguousarray(arr2d).reshape(nch, P, fd)
    parts = [a[b0:b0 + min(cpb, nch - b0)].transpose(1, 0, 2).reshape(P, -1)
             for b0 in range(0, nch, cpb)]
    return np.ascontiguousarray(np.concatenate(parts, axis=1))


def kernel(output, trg, lengths, _trace=False, _tmpdir=None):
    import ml_dtypes
    from concourse.bass_utils import run_bass_kernel_spmd

    output = np.asarray(output, dtype=np.float32)
    assert output.shape == (B, SP1, V)
    trg = np.asarray(trg)
    lengths = np.asarray(lengths)

    L = np.clip(lengths.astype(np.int64), 0, S)
    tgt = trg[:, 1:].astype(np.int64)                    # [B, S]

    b_idx = np.repeat(np.arange(B), L)
    k_idx = np.concatenate([np.arange(n) for n in L]) if L.sum() else \
        np.zeros(0, np.int64)
    n_valid = b_idx.shape[0]
    if n_valid == 0:
        return np.float32(0.0)

    n_real = -(-n_valid // NCORES)                       # tokens per core
    T_f = n_real // P                                    # full ACT tiles
    va, nch_b, tail, nch_c = _geom(T_f, n_real)

    flat = output.reshape(B * SP1, V)
    row_ids = b_idx * SP1 + 1 + k_idx
    pad = NCORES * n_real - n_valid                      # <= 7
    row_ids_p = np.concatenate([row_ids, np.full(pad, row_ids[0])])

    x8 = flat[row_ids_p].astype(ml_dtypes.float8_e4m3)   # [8*n_real, 32000]
    in_maps = []
    for m in range(NCORES):
        blk = x8[m * n_real:(m + 1) * n_real]
        im = {"xb": _blockify(blk[:, va:].T, nch_b, CPB, n_real)}
        if T_f:
            im["xa"] = np.ascontiguousarray(blk[:T_f * P, :va])
        if nch_c:
            im["xc"] = _blockify(blk[T_f * P:, :va].T, nch_c, CPB_C, tail)
        in_maps.append(im)

    nc = _get_program(T_f, n_real)
    res = run_bass_kernel_spmd(nc, in_maps, core_ids=list(range(NCORES)),
                               trace=_trace, tmpdir=_tmpdir)

    se = np.empty(NCORES * n_real, dtype=np.float64)
    for m in range(NCORES):
        sb = res.results[m]["sb"][0].astype(np.float64)
        if T_f:
            sa = res.results[m]["sa"].T.reshape(T_f * P).astype(np.float64)
            sb[:T_f * P] += sa
        se[m * n_real:(m + 1) * n_real] = sb
    lse = np.log(se[:n_valid])

    tgt_tok = tgt[b_idx, k_idx]
    x_tgt = flat[row_ids, tgt_tok]
    keep = tgt_tok != 0                                  # ignore_index=0
    nll = (lse - x_tgt.astype(np.float64)) * keep
    denom = max(float(keep.sum()), 1.0)
    loss = nll.sum() / denom
    out = np.float32(loss)
    if _trace:
        return out, res
    return out


# revision 21
# speedup vs baseline: 1.0287x; 1.0287x over previous
"""Masked cross-entropy loss (ragged sequences) on 8 Trainium2 NeuronCores.

loss = sum_{valid} (logsumexp_v(logits[b,s,:]) - logits[b,s,tgt]) / n_valid
where valid = (position k < lengths[b]) & (tgt != 0), logits = output[:, 1:].

The heavy work is the per-token sum_v exp(x_v) over the 32000-wide vocab.
Valid tokens are packed host-side and shipped as fp8-e4m3 (halves HBM
traffic); the vocab is column-split across two concurrent engine paths per
core so no single engine is the bottleneck:

  * ACT path (V_A cols, full 128-token tiles): ScalarE activation Exp with
    per-partition accumulate, layout [tokens(128p), vocab(free)].
  * DVE+PE path (V_B cols, all tokens; plus the partial-tile tail tokens'
    V_A cols): layout [vocab(128p), tokens(free)]. VectorE computes a
    Schraudolph bit-trick exp in ONE tensor_scalar pass:
    i16 = round(x*1024/ln2 + (15*1024 + C)); those bits reinterpreted as
    fp16 ARE ~exp(x) (C calibrated to make the mean ratio 1). TensorE sums
    over the 128 vocab partitions with a ones[128,1] stationary matmul
    accumulating into PSUM[1, tokens] across all vocab chunks.

Host adds the two partial sums per token, takes log, gathers target logits
from the f32 data, masks (ignore_index=0) and reduces. Device traffic is
n_tokens*32000 bytes/core, streamed via a few dozen fat contiguous DMAs.
"""

import numpy as np

B, SP1, V = 16, 513, 32000
S = SP1 - 1
NCORES = 8
P = 128

V_A = 16128                # ACT-path vocab columns (when full tiles exist)
ACT_RAMP = [1536, 2048, 2560, 3072, 3456, 3456]   # tile-0 chunks, sum V_A
ACT_CW = 8064              # steady-tile chunk width (2 per tile)
CPB = 7                    # vocab chunks per DVE/DMA block (xb path)
CPB_C = 32                 # chunks per block for the tail (xc) path
PSUM_FD = 512

TRICK_A = 1024.0 / float(np.log(2.0))
TRICK_B = 1024.0 * 15 - 59.143        # exponent bias + calibration

_programs = {}             # (T_f, n_real) -> compiled Bacc program


def _geom(T_f, n_real):
    va = V_A if T_f > 0 else 0
    nch_b = (V - va) // P                  # V - va is a multiple of 128
    tail = n_real - T_f * P
    nch_c = va // P if tail else 0
    return va, nch_b, tail, nch_c


def _build_program(T_f, n_real):
    import concourse.bacc as bacc
    import concourse.tile as tile
    from concourse import mybir

    va, nch_b, tail, nch_c = _geom(T_f, n_real)
    assert T_f * P + tail == n_real and 0 <= tail < P

    nc = bacc.Bacc("TRN2", target_bir_lowering=False, debug=False,
                   num_devices=NCORES)
    xb = nc.dram_tensor("xb", [P, nch_b * n_real], mybir.dt.float8e4,
                        kind="ExternalInput").ap()
    xa = xc = sa = None
    if T_f:
        xa = nc.dram_tensor("xa", [T_f * P, va], mybir.dt.float8e4,
                            kind="ExternalInput").ap()
        sa = nc.dram_tensor("sa", [P, T_f], mybir.dt.float32,
                            kind="ExternalOutput").ap()
    if nch_c:
        xc = nc.dram_tensor("xc", [P, nch_c * tail], mybir.dt.float8e4,
                            kind="ExternalInput").ap()
    sb = nc.dram_tensor("sb", [1, n_real], mybir.dt.float32,
                        kind="ExternalOutput").ap()

    g1 = min(n_real, PSUM_FD)
    groups = [(0, g1)] + ([(g1, n_real - g1)] if g1 < n_real else [])
    assert sum(ACT_RAMP) == V_A and 2 * ACT_CW == V_A
    sched = [ACT_RAMP if j == 0 else [ACT_CW, ACT_CW] for j in range(T_f)]
    # matmul counts per PSUM group, for start/stop flags
    writes = [nch_b + (nch_c if g0 + gw > T_f * P else 0)
              for (g0, gw) in groups]
    seen = [0] * len(groups)

    with tile.TileContext(nc) as tc:
        with (
            tc.tile_pool(name="xpa", bufs=4) as xpa,
            tc.tile_pool(name="scr", bufs=2) as scr,
            tc.tile_pool(name="sm", bufs=8) as sm,
            tc.tile_pool(name="xpb", bufs=6) as xpb,
            tc.tile_pool(name="itp", bufs=4) as itp,
            tc.tile_pool(name="one", bufs=1) as one,
            tc.tile_pool(name="ps", bufs=1, space="PSUM") as ps,
        ):
            ones = one.tile([P, 1], mybir.dt.float16)
            nc.vector.memset(ones, 1.0)
            totalA = one.tile([P, max(T_f, 1)], mybir.dt.float32)
            seB_sb = one.tile([1, n_real], mybir.dt.float32)
            psums = [ps.tile([1, gw], mybir.dt.float32, tag=f"ps{g}",
                             name=f"psum{g}")
                     for g, (_, gw) in enumerate(groups)]

            def mm(g, rhs, goff, gwid):
                """Accumulating matmul into group g's psum tile."""
                seen[g] += 1
                nc.tensor.matmul(psums[g][:, goff:goff + gwid], ones, rhs,
                                 start=(seen[g] == 1),
                                 stop=(seen[g] == writes[g]))

            sums_tiles = []

            def act_steps():
                for j in range(T_f):
                    chunks = sched[j]
                    sums = sm.tile([P, len(chunks)], mybir.dt.float32,
                                   tag=f"sums{j}", name=f"sums{j}")
                    sums_tiles.append((sums, len(chunks)))
                    off = 0
                    for c, cw in enumerate(chunks):
                        xt = xpa.tile([P, cw], mybir.dt.float8e4, tag="xt")
                        # The first two DMAs go out on the ScalarE hwdge
                        # queue: it is idle during the sync engine's startup
                        # boilerplate, so ACT gets fed ~3us earlier.
                        eng = nc.scalar if j == 0 and c < 2 else nc.sync
                        eng.dma_start(
                            out=xt, in_=xa[j * P:(j + 1) * P,
                                           off:off + cw])
                        et = scr.tile([P, cw], mybir.dt.float8e4, tag="scr")
                        nc.scalar.activation(
                            et, xt, mybir.ActivationFunctionType.Exp,
                            accum_out=sums[:, c:c + 1])
                        off += cw
                        yield cw * P

            def xb_block(b0):
                k = min(CPB, nch_b - b0)
                xt = xpb.tile([P, k * n_real], mybir.dt.float8e4, tag="xb")
                nc.sync.dma_start(
                    out=xt, in_=xb[:, b0 * n_real:(b0 + k) * n_real])
                it = itp.tile([P, k * n_real], mybir.dt.int16, tag="it")
                nc.vector.tensor_scalar(
                    it, xt, TRICK_A, TRICK_B,
                    mybir.AluOpType.mult, mybir.AluOpType.add)
                ft = it.bitcast(mybir.dt.float16)
                for i in range(k):
                    for g, (g0, gw) in enumerate(groups):
                        mm(g, ft[:, i * n_real + g0:i * n_real + g0 + gw],
                           0, gw)
                return k * n_real * P

            def xc_block(b0):
                # tail tokens' ACT-share columns, [vocab(128p), tail] chunks
                k = min(CPB_C, nch_c - b0)
                xt = xpb.tile([P, k * tail], mybir.dt.float8e4, tag="xc")
                nc.sync.dma_start(
                    out=xt, in_=xc[:, b0 * tail:(b0 + k) * tail])
                it = itp.tile([P, k * tail], mybir.dt.int16, tag="itc")
                nc.vector.tensor_scalar(
                    it, xt, TRICK_A, TRICK_B,
                    mybir.AluOpType.mult, mybir.AluOpType.add)
                ft = it.bitcast(mybir.dt.float16)
                goff = T_f * P - groups[-1][0]
                for i in range(k):
                    mm(len(groups) - 1, ft[:, i * tail:(i + 1) * tail],
                       goff, tail)
                return k * tail * P

            def dve_steps():
                # interleave the small xc blocks evenly among the xb blocks
                items = [("b", b0) for b0 in range(0, nch_b, CPB)]
                nb = len(items)
                ncb = -(-nch_c // CPB_C) if nch_c else 0
                for i in range(ncb, 0, -1):
                    items.insert(i * nb // (ncb + 1) + 1,
                                 ("c", (i - 1) * CPB_C))
                for kind, b0 in items:
                    yield xb_block(b0) if kind == "b" else xc_block(b0)

            ga, gb = act_steps(), dve_steps()
            ba = bb = 0.0
            a_done = b_done = False
            while not (a_done and b_done):
                if b_done or (not a_done and ba <= bb):
                    try:
                        ba += next(ga)
                    except StopIteration:
                        a_done = True
                else:
                    try:
                        bb += next(gb)
                    except StopIteration:
                        b_done = True

            # Deferred ACT-tile reduces: emitted after all DVE work so the
            # ACT->reduce dependency never stalls mid-stream DVE ops.
            for j, (sums, nch) in enumerate(sums_tiles):
                nc.vector.tensor_reduce(
                    out=totalA[:, j:j + 1], in_=sums[:, :nch],
                    axis=mybir.AxisListType.X, op=mybir.AluOpType.add)
            if T_f:
                nc.sync.dma_start(out=sa, in_=totalA)
            for g, (g0, gw) in enumerate(groups):
                nc.vector.tensor_copy(seB_sb[:, g0:g0 + gw], psums[g])
            nc.sync.dma_start(out=sb, in_=seB_sb)

    nc.compile()
    return nc


def _get_program(T_f, n_real):
    key = (T_f, n_real)
    if key not in _programs:
        _programs[key] = _build_program(T_f, n_real)
    return _programs[key]


def _blockify(arr2d, nch, cpb, fd):
    """[nch*P, fd] -> [P, nch*fd] with cpb-chunk blocks laid contiguously."""
    a = np.ascontiguousarray(arr2d).reshape(nch, P, fd)
    parts = [a[b0:b0 + min(cpb, nch - b0)].transpose(1, 0, 2).reshape(P, -1)
             for b0 in range(0, nch, cpb)]
    return np.ascontiguousarray(np.concatenate(parts, axis=1))


def kernel(output, trg, lengths, _trace=False, _tmpdir=None):
    import ml_dtypes
    from concourse.bass_utils import run_bass_kernel_spmd

    output = np.asarray(output, dtype=np.float32)
    assert output.shape == (B, SP1, V)
    trg = np.asarray(trg)
    lengths = np.asarray(lengths)

    L = np.clip(lengths.astype(np.int64), 0, S)
    tgt = trg[:, 1:].astype(np.int64)                    # [B, S]

    b_idx = np.repeat(np.arange(B), L)
    k_idx = np.concatenate([np.arange(n) for n in L]) if L.sum() else \
        np.zeros(0, np.int64)
    n_valid = b_idx.shape[0]
    if n_valid == 0:
        return np.float32(0.0)

    n_real = -(-n_valid // NCORES)                       # tokens per core
    T_f = n_real // P                                    # full ACT tiles
    va, nch_b, tail, nch_c = _geom(T_f, n_real)

    flat = output.reshape(B * SP1, V)
    row_ids = b_idx * SP1 + 1 + k_idx
    pad = NCORES * n_real - n_valid                      # <= 7
    row_ids_p = np.concatenate([row_ids, np.full(pad, row_ids[0])])

    x8 = flat[row_ids_p].astype(ml_dtypes.float8_e4m3)   # [8*n_real, 32000]
    in_maps = []
    for m in range(NCORES):
        blk = x8[m * n_real:(m + 1) * n_real]
        im = {"xb": _blockify(blk[:, va:].T, nch_b, CPB, n_real)}
        if T_f:
            im["xa"] = np.ascontiguousarray(blk[:T_f * P, :va])
        if nch_c:
            im["xc"] = _blockify(blk[T_f * P:, :va].T, nch_c, CPB_C, tail)
        in_maps.append(im)

    nc = _get_program(T_f, n_real)
    res = run_bass_kernel_spmd(nc, in_maps, core_ids=list(range(NCORES)),
                               trace=_trace, tmpdir=_tmpdir)

    se = np.empty(NCORES * n_real, dtype=np.float64)
    for m in range(NCORES):
        sb = res.results[m]["sb"][0].astype(np.float64)
        if T_f:
            sa = res.results[m]["sa"].T.reshape(T_f * P).astype(np.float64)
            sb[:T_f * P] += sa
        se[m * n_real:(m + 1) * n_real] = sb
    lse = np.log(se[:n_valid])

    tgt_tok = tgt[b_idx, k_idx]
    x_tgt = flat[row_ids, tgt_tok]
    keep = tgt_tok != 0                                  # ignore_index=0
    nll = (lse - x_tgt.astype(np.float64)) * keep
    denom = max(float(keep.sum()), 1.0)
    loss = nll.sum() / denom
    out = np.float32(loss)
    if _trace:
        return out, res
    return out
